# revision 15
# baseline (speedup 1.0000x reference)
"""Trainium2 Bass kernel for a 2-layer GCN (segment-sum aggregation).

out = softmax( A @ relu(A @ h @ W1 + b1) @ W2 + b2 ),  A = adjacency (+self loops)

Strategy (8 NeuronCores, node/data parallel):
  - Nodes sharded by range: core k owns nodes [k*12500, (k+1)*12500).
  - Edges routed (on host) to the core owning their dst node, grouped by
    (128-node dst window, src%4 phase). Slot sizes are the max count over the
    8 cores rounded to 32 (SPMD: one program serves all cores); slots are
    packed contiguously per (group, phase) run, so chunks may straddle slot
    boundaries (the one-hot masks foreign positions with -1000).
  - Layer 1 on device: dma_gather of h[src] rows (256B) from an HBM table
    viewed as [N/4, 1024B] super-rows (dodges the signed-int16 index limit);
    per 128-edge chunk a one-hot [edge x node] matrix (built batched, one
    DVE tensor_tensor is_equal per window) and the TensorEngine accumulates
    aggT = sum featsT @ onehot into PSUM per window (segment-sum).
    Self loops are a dense add.  Then x1T = relu(W1.T @ aggT + b1) and
    y = x1T.T @ W2 (transform BEFORE layer-2 aggregation: 128 -> 40 dims).
  - AllGather of per-core y slices -> full y table (same super-row layout).
    b2 is folded into the local yown copy while the AllGather runs.
  - Layer 2: same gathers/one-hots vs the y table, orientation flipped to
    give node-major [128,64] windows; + self loop (incl b2); softmax on chip
    (Exp with accum_out row-sum + fast approx reciprocal).
"""

import math
import numpy as np

D = 64          # input feature dim (one gather row = 256B)
HID = 128
C = 40
CORES = 8
WIN = 128       # dst window (nodes per one-hot matmul window)
NPHASE = 4      # src mod-4 phases (int16 gather index reach)
GROUP = 4       # windows per gather instruction group
SLOT_GRAN = 32  # slot size granularity (num_idxs must be %16)
MAXIDX = 4096   # max idxs per dma_gather (single_packet=False)
# AllGather split points (stage-A group index after which each part fires).
# Parts must be window-aligned; the last part is fired after stage A.
CC_GROUPS = (12, 20)


def _cc_parts(nloc):
    """Per-part (row_lo, row_hi, sr, y4_row_off) for the split AllGather."""
    bounds = [0] + [(g + 1) * GROUP * WIN for g in CC_GROUPS] + [nloc]
    parts = []
    off = 0
    for lo, hi in zip(bounds[:-1], bounds[1:]):
        sr = (hi - lo) // NPHASE
        parts.append((lo, hi, sr, off))
        off += CORES * sr
    return parts


def _remap_idx(v, nloc):
    """Map global super-row v (= core*nloc/4 + r) to its row in the
    part-major y4 table produced by the split AllGather."""
    srl = nloc // NPHASE
    c = v // srl
    r = v % srl
    out = np.zeros_like(v)
    for lo, hi, sr, off in _cc_parts(nloc):
        lo4, hi4 = lo // NPHASE, hi // NPHASE
        m = (r >= lo4) & (r < hi4)
        out[m] = off + c[m] * sr + (r[m] - lo4)
    return out


# ----------------------------------------------------------------------------
# Host-side routing
# ----------------------------------------------------------------------------

def route_edges(src, dst, n_nodes, cores=CORES):
    """Group edges by (core, window, phase) into contiguous runs per
    (group, phase).  Slot sizes are shared across cores (max, %32); the
    edge->position layout is identical for all cores (SPMD program)."""
    nloc = n_nodes // cores
    nw = math.ceil(nloc / WIN)
    ngroups = math.ceil(nw / GROUP)
    src = src.astype(np.int64)
    dst = dst.astype(np.int64)
    core = dst // nloc
    dloc = dst % nloc
    w = dloc // WIN
    phi = src % NPHASE
    key = (w * NPHASE + phi).astype(np.int64)   # per-core key in [0, nw*4)

    counts = np.zeros((cores, nw * NPHASE), np.int64)
    for k in range(cores):
        counts[k] = np.bincount(key[core == k], minlength=nw * NPHASE)
    nmax = counts.max(axis=0)                   # [nw*4]
    S = ((nmax + SLOT_GRAN - 1) // SLOT_GRAN * SLOT_GRAN).astype(np.int64)
    # ensure every window has at least one slot (self-loop-only windows)
    for wi in range(nw):
        if S[wi * NPHASE:(wi + 1) * NPHASE].sum() == 0:
            S[wi * NPHASE] = SLOT_GRAN

    def group_windows(g):
        return range(g * GROUP, min((g + 1) * GROUP, nw))

    # --- run layout: runs are (g, j) with j = p>>1 (phases 2j, 2j+1 share
    # the gather in_ap column), slots contiguous within the run ---
    offs = np.zeros(nw * NPHASE, np.int64)   # slot -> global idx-stream offset
    a_rel = np.zeros(nw * NPHASE, np.int64)  # slot -> offset within its run
    slot_col0 = np.zeros(nw * NPHASE, np.int64)  # slot -> run chunk col base
    run_len = {}                             # (g, j) -> total idxs
    run_off = {}                             # (g, j) -> global idx offset
    col0 = {}                                # (g, j) -> first chunk col in gbuf
    gchunks = []                             # chunk cols per group buffer
    tot = 0
    for g in range(ngroups):
        col = 0
        for j in range(NPHASE // 2):
            run_off[(g, j)] = tot
            col0[(g, j)] = col
            acc = 0
            for p in (2 * j, 2 * j + 1):
                for wi in group_windows(g):
                    s = wi * NPHASE + p
                    offs[s] = tot + acc
                    a_rel[s] = acc
                    slot_col0[s] = col
                    acc += int(S[s])
            run_len[(g, j)] = acc
            tot += acc
            col += (acc + WIN - 1) // WIN
        gchunks.append(col)

    # --- per-window one-hot column allocation (all 4 phases contiguous) ---
    # chunks[(w)] = list of (dstcol, gbuf_chunk_col, half) in matmul order
    dcol = 0
    dcol0 = np.zeros(nw, np.int64)
    wspan = np.zeros(nw, np.int64)
    chunks = {w: [] for w in range(nw)}
    # map: for each dst column, (slot, c_global) to fill host-side values
    dstcol_fill = []   # list of (slot, run-relative chunk index c)
    for g in range(ngroups):
        for wi in group_windows(g):
            dcol0[wi] = dcol
            for p in range(NPHASE):
                s = wi * NPHASE + p
                if S[s] == 0:
                    continue
                a = int(a_rel[s])
                c_lo = a // WIN
                c_hi = (a + int(S[s]) + WIN - 1) // WIN
                for c in range(c_lo, c_hi):
                    chunks[wi].append((dcol, int(slot_col0[s]) + c, p & 1))
                    dstcol_fill.append((s, c))
                    dcol += 1
            wspan[wi] = dcol - dcol0[wi]
    ndcols = dcol

    # --- per-core streams ---
    idx_streams, dst_streams = [], []
    for k in range(cores):
        sel = core == k
        kk = key[sel]
        sidx = np.argsort(kk, kind="stable")
        kk_s = kk[sidx]
        occ = np.arange(len(kk_s)) - np.repeat(
            np.r_[0, np.cumsum(np.bincount(kk_s, minlength=nw * NPHASE))[:-1]][kk_s], 1)
        pos = offs[kk_s] + occ
        idx = np.zeros(tot, np.int16)           # pad: super-row 0 (valid)
        idx[pos] = (src[sel][sidx] >> 2).astype(np.int16)
        # dst one-hot values, laid out per allocated dst column
        dsl_slot = np.full(tot, -1000.0, np.float32)  # per idx position
        dsl_slot[pos] = (dloc[sel][sidx] % WIN).astype(np.float32)
        dstv = np.full((ndcols, WIN), -1000.0, np.float32)
        for dc, (s, c) in enumerate(dstcol_fill):
            a = int(a_rel[s])
            g = -1  # recover run offset
            # run-relative position range of this chunk: [c*WIN, (c+1)*WIN)
            # slot covers run-relative [a, a+S[s])
            lo = max(c * WIN, a)
            hi = min((c + 1) * WIN, a + int(S[s]))
            # global idx positions: offs[s] + (i - a) for i in [lo, hi)
            qs = np.arange(lo, hi)
            dstv[dc, qs % WIN] = dsl_slot[offs[s] + (qs - a)]
        idx_streams.append(idx)
        dst_streams.append(dstv.T.copy())       # [WIN, ndcols]
    return dict(S=S, tot=tot, nw=nw, nloc=nloc, ngroups=ngroups,
                run_len=run_len, run_off=run_off, col0=col0, gchunks=gchunks,
                dcol0=dcol0, wspan=wspan, chunks=chunks, ndcols=ndcols,
                idx=idx_streams, dst=dst_streams)


# ----------------------------------------------------------------------------
# Bass program
# ----------------------------------------------------------------------------

def build_program(n_nodes, rt, do_cc=True, split_cc=True):
    import concourse.bass as bass
    import concourse.mybir as mybir
    import concourse.bacc as bacc
    from concourse import tile

    f32 = mybir.dt.float32
    bf16 = mybir.dt.float16
    i16 = mybir.dt.int16
    tot, nw, nloc, ngroups = rt["tot"], rt["nw"], rt["nloc"], rt["ngroups"]
    run_len, run_off, col0 = rt["run_len"], rt["run_off"], rt["col0"]
    gchunks, dcol0, wspan, chunks = (rt["gchunks"], rt["dcol0"], rt["wspan"],
                                     rt["chunks"])
    ndcols = rt["ndcols"]
    nsup = n_nodes // NPHASE               # super-rows in gather tables
    nlocp = nw * WIN                       # padded local node count
    last_rows = nloc - (nw - 1) * WIN      # rows in the last window
    gcmax = max(gchunks)                   # gather buffer chunk cols
    spanmax = int(max(wspan))              # widest per-window one-hot

    nc = bacc.Bacc(None, target_bir_lowering=False, debug=False,
                   num_swdge_queues=4)

    h4 = nc.declare_dram_parameter("h4", [nsup, NPHASE * D], bf16, False)
    hTo = nc.declare_dram_parameter("hTo", [D, nlocp], f32, False)
    W1d = nc.declare_dram_parameter("W1", [D, HID], f32, False)
    b1d = nc.declare_dram_parameter("b1", [HID, 1], f32, False)
    W2d = nc.declare_dram_parameter("W2p", [HID, D], f32, False)
    b2d = nc.declare_dram_parameter("b2b", [WIN, D], f32, False)
    idxd = nc.declare_dram_parameter("idx", [128, tot // 16], i16, False)
    idxd2 = nc.declare_dram_parameter("idx2", [128, tot // 16], i16, False)
    dstd = nc.declare_dram_parameter("dstf", [WIN, ndcols], bf16, False)
    iotad = nc.declare_dram_parameter("iota", [WIN, spanmax * WIN], bf16, False)
    outd = nc.declare_dram_parameter("out", [nloc, C], f32, True)

    cc_in = nc.dram_tensor("cc_in", [nloc, D], bf16)
    y4 = nc.dram_tensor("y4", [CORES * nloc // NPHASE, NPHASE * D], bf16,
                        addr_space="Shared")

    def group_windows(g):
        return range(g * GROUP, min((g + 1) * GROUP, nw))

    Relu = mybir.ActivationFunctionType.Relu
    Exp = mybir.ActivationFunctionType.Exp
    add_op = mybir.AluOpType.add
    eq_op = mybir.AluOpType.is_equal

    with tile.TileContext(nc) as tc:
        import contextlib
        with contextlib.ExitStack() as ctx:
            cpool = ctx.enter_context(tc.tile_pool(name="const", bufs=1))
            ypool = ctx.enter_context(tc.tile_pool(name="yown", bufs=1))

            idx_sb = cpool.tile([128, tot // 16], i16)
            dst_sb = cpool.tile([WIN, ndcols], bf16)
            iota_sb = cpool.tile([WIN, spanmax * WIN], bf16)
            hTo_sb = cpool.tile([D, nlocp], f32)
            W1_sb = cpool.tile([D, HID], f32)
            b1_sb = cpool.tile([HID, 1], f32)
            W2_sb = cpool.tile([HID, D], f32)
            b2_sb = cpool.tile([WIN, D], f32)
            yown = ypool.tile([WIN, nw * D], f32)

            nc.sync.dma_start(idx_sb[:], idxd[:])
            nc.sync.dma_start(dst_sb[:], dstd[:])
            nc.sync.dma_start(iota_sb[:], iotad[:])
            nc.sync.dma_start(hTo_sb[:], hTo[:])
            nc.sync.dma_start(W1_sb[:], W1d[:])
            nc.sync.dma_start(b1_sb[:], b1d[:])
            nc.sync.dma_start(W2_sb[:], W2d[:])
            nc.sync.dma_start(b2_sb[:], b2d[:])

            qctr = [0]      # round-robin SWDGE queue (4 Q7 core pairs)

            def issue_gathers(g, gt, table):
                for j in range(NPHASE // 2):
                    n = run_len[(g, j)]
                    if n == 0:
                        continue
                    o = run_off[(g, j)]
                    c0 = col0[(g, j)]
                    for s0 in range(0, n, MAXIDX):
                        ni = min(MAXIDX, n - s0)
                        cc0 = c0 + s0 // WIN
                        nch_i = (ni + WIN - 1) // WIN
                        oo = o + s0
                        nc.gpsimd.dma_gather(
                            out_ap=gt[:, cc0 * 2 * D:(cc0 + nch_i) * 2 * D]
                                .rearrange("p (c f) -> p c f", f=2 * D),
                            in_ap=table[:, j * 2 * D:(j + 1) * 2 * D],
                            idxs_ap=idx_sb[:, oo // 16: (oo + ni) // 16],
                            num_idxs=ni,
                            num_idxs_reg=ni,
                            elem_size=2 * D,
                            elem_step=NPHASE * D,
                            single_packet=False,
                            queue_num=qctr[0] % 4,
                        )
                        qctr[0] += 1

            def build_onehot(ohpool, wi):
                span = int(wspan[wi])
                d0 = int(dcol0[wi])
                oh = ohpool.tile([WIN, spanmax * WIN], bf16)
                nc.vector.tensor_tensor(
                    oh[:, :span * WIN].rearrange("p (c f) -> p c f", f=WIN),
                    iota_sb[:, :span * WIN].rearrange("p (c f) -> p c f", f=WIN),
                    dst_sb[:, d0:d0 + span].unsqueeze(2)
                        .broadcast_to((WIN, span, WIN)),
                    eq_op)
                return oh

            # split AllGather: early parts fire mid stage-A so their wire
            # time hides under layer-1 compute.  The y4 table is laid out
            # part-major (each part's AllGather output is contiguous); the
            # layer-2 idx stream (idx2) is remapped host-side to match.
            cc_parts = _cc_parts(nloc)
            cc_after = {CC_GROUPS[i]: cc_parts[i] for i in range(len(CC_GROUPS))}

            def fire_cc(part):
                lo, hi, sr, off = part
                nc.gpsimd.collective_compute(
                    "AllGather", mybir.AluOpType.bypass,
                    replica_groups=[list(range(CORES))],
                    ins=[cc_in.ap()[lo:hi, :]],
                    outs=[y4.ap()[off:off + CORES * sr, :]])

            # ---------------- stage A: layer 1 ----------------
            with contextlib.ExitStack() as sa:
                gpool = sa.enter_context(tc.tile_pool(name="gatherA", bufs=3))
                ohpool = sa.enter_context(tc.tile_pool(name="ohA", bufs=8))
                aggpool = sa.enter_context(tc.tile_pool(name="aggT", bufs=4))
                xpool = sa.enter_context(tc.tile_pool(name="x1", bufs=4))
                psA = sa.enter_context(
                    tc.tile_pool(name="psA", bufs=3, space="PSUM"))
                psB = sa.enter_context(
                    tc.tile_pool(name="psB", bufs=2, space="PSUM"))
                psC = sa.enter_context(
                    tc.tile_pool(name="psC", bufs=2, space="PSUM"))

                for g in range(ngroups):
                    gt = gpool.tile([WIN, gcmax * 2 * D], bf16, tag="gbuf")
                    if g < 3:
                        nc.vector.memset(gt[:], 0.0)
                    issue_gathers(g, gt, h4)
                    for wi in group_windows(g):
                        ch = chunks[wi]
                        oh = build_onehot(ohpool, wi)
                        d0 = int(dcol0[wi])
                        ps = psA.tile([D, WIN], f32)
                        for i, (dc, gcol, half) in enumerate(ch):
                            c0 = gcol * 2 * D + half * D
                            ci = dc - d0
                            nc.tensor.matmul(
                                ps[:], gt[:, c0:c0 + D],
                                oh[:, ci * WIN:(ci + 1) * WIN],
                                start=(i == 0), stop=(i == len(ch) - 1))
                        aggT = aggpool.tile([D, WIN], f32)
                        nc.vector.tensor_tensor(
                            aggT[:], ps[:], hTo_sb[:, wi * WIN:(wi + 1) * WIN],
                            add_op)
                        ps2 = psB.tile([HID, WIN], f32)
                        nc.tensor.matmul(ps2[:], W1_sb[:], aggT[:])
                        x1 = xpool.tile([HID, WIN], f32)
                        nc.scalar.activation(x1[:], ps2[:], Relu,
                                             bias=b1_sb[:, 0:1])
                        ps3 = psC.tile([WIN, D], f32)
                        nc.tensor.matmul(ps3[:], x1[:], W2_sb[:])
                        nc.scalar.copy(yown[:, wi * D:(wi + 1) * D], ps3[:])
                        ybf = xpool.tile([WIN, D], bf16, tag="ybf")
                        nc.scalar.copy(ybf[:], ps3[:])
                        rows = last_rows if wi == nw - 1 else WIN
                        nc.sync.dma_start(
                            cc_in[wi * WIN: wi * WIN + rows, :],
                            ybf[:rows, :])
                    if do_cc and split_cc and g in cc_after:
                        fire_cc(cc_after[g])

            # ---------------- all-gather of y ----------------
            if do_cc:
                if split_cc:
                    fire_cc(cc_parts[-1])
                else:
                    nc.gpsimd.collective_compute(
                        "AllGather", mybir.AluOpType.bypass,
                        replica_groups=[list(range(CORES))],
                        ins=[cc_in.ap().opt()], outs=[y4.ap().opt()])

            # swap in the layer-2 index stream (remapped for the part-major
            # y4 layout); runs in the AllGather bubble
            nc.sync.dma_start(idx_sb[:], idxd2[:])

            # fold b2 into the self-loop copy while the AllGather runs
            nc.vector.tensor_tensor(
                yown[:].rearrange("p (w d) -> p w d", d=D),
                yown[:].rearrange("p (w d) -> p w d", d=D),
                b2_sb[:].unsqueeze(1).broadcast_to((WIN, nw, D)),
                add_op)

            # ---------------- stage C: layer 2 ----------------
            with contextlib.ExitStack() as sc:
                gpool = sc.enter_context(tc.tile_pool(name="gatherC", bufs=3))
                ohpool = sc.enter_context(tc.tile_pool(name="ohC", bufs=8))
                spool = sc.enter_context(tc.tile_pool(name="smax", bufs=4))
                opool = sc.enter_context(tc.tile_pool(name="outp", bufs=3))
                psD = sc.enter_context(
                    tc.tile_pool(name="psD", bufs=4, space="PSUM"))

                for g in range(ngroups):
                    gt = gpool.tile([WIN, gcmax * 2 * D], bf16, tag="gbufC")
                    if g < 3:
                        nc.vector.memset(gt[:], 0.0)
                    issue_gathers(g, gt, y4)
                    for wi in group_windows(g):
                        ch = chunks[wi]
                        oh = build_onehot(ohpool, wi)
                        d0 = int(dcol0[wi])
                        ps = psD.tile([WIN, D], f32)
                        for i, (dc, gcol, half) in enumerate(ch):
                            c0 = gcol * 2 * D + half * D
                            ci = dc - d0
                            nc.tensor.matmul(
                                ps[:], oh[:, ci * WIN:(ci + 1) * WIN],
                                gt[:, c0:c0 + D],
                                start=(i == 0), stop=(i == len(ch) - 1))
                        t1 = spool.tile([WIN, D], f32, tag="t1")
                        nc.vector.tensor_tensor(
                            t1[:], ps[:], yown[:, wi * D:(wi + 1) * D], add_op)
                        mx = spool.tile([WIN, 1], f32, tag="mx")
                        nc.vector.tensor_reduce(
                            mx[:], t1[:, :C], mybir.AxisListType.X,
                            mybir.AluOpType.max, negate=True)
                        e = spool.tile([WIN, C], f32, tag="e")
                        sm = spool.tile([WIN, 1], f32, tag="sm")
                        nc.scalar.activation(e[:], t1[:, :C], Exp,
                                             bias=mx[:, 0:1],
                                             accum_out=sm[:, 0:1])
                        ri = spool.tile([WIN, 1], f32, tag="ri")
                        nc.vector.reciprocal_approx_fast(ri[:], sm[:])
                        o = opool.tile([WIN, C], f32)
                        nc.scalar.activation(
                            o[:], e[:], mybir.ActivationFunctionType.Identity,
                            scale=ri[:, 0:1])
                        rows = last_rows if wi == nw - 1 else WIN
                        nc.sync.dma_start(
                            outd[wi * WIN: wi * WIN + rows, :], o[:rows, :])

    nc.finalize()
    return nc


# ----------------------------------------------------------------------------
# Entry point
# ----------------------------------------------------------------------------

def _prepare_inputs(node_embeddings, adjacency_lists, W1, b1, W2, b2, rt):
    n, d = node_embeddings.shape
    nloc, nw = rt["nloc"], rt["nw"]
    nlocp = nw * WIN
    spanmax = int(max(rt["wspan"]))
    bf = np.float16
    h = np.ascontiguousarray(node_embeddings, np.float32)
    h4 = h.astype(bf).reshape(n // NPHASE, NPHASE * d)
    W2p = np.zeros((HID, D), np.float32)
    W2p[:, :C] = W2
    b2b = np.tile(np.pad(b2.astype(np.float32), (0, D - C)), (WIN, 1))
    iota = np.tile(np.arange(WIN, dtype=np.float32), (WIN, spanmax))
    in_maps = []
    for k in range(CORES):
        hTo = np.zeros((d, nlocp), np.float32)
        hTo[:, :nloc] = h[k * nloc:(k + 1) * nloc].T
        in_maps.append({
            "h4": h4,
            "hTo": hTo,
            "W1": np.ascontiguousarray(W1, np.float32),
            "b1": np.ascontiguousarray(b1, np.float32).reshape(HID, 1),
            "W2p": W2p,
            "b2b": b2b,
            "idx": np.tile(rt["idx"][k].reshape(-1, 16).T, (8, 1)).copy(),
            "idx2": np.tile(
                _remap_idx(rt["idx"][k].astype(np.int64), nloc)
                .astype(np.int16).reshape(-1, 16).T, (8, 1)).copy(),
            "dstf": np.ascontiguousarray(rt["dst"][k]).astype(bf),
            "iota": iota.astype(bf),
            "out": np.zeros((nloc, C), np.float32),
        })
    return in_maps


_CACHE = {}


def _get_program(n_nodes, rt_sig, rt):
    key = (n_nodes, rt_sig)
    if key not in _CACHE:
        _CACHE[key] = build_program(n_nodes, rt)
    return _CACHE[key]


def kernel(node_embeddings, adjacency_lists, W1, b1, W2, b2, trace=False):
    import sys
    if "/opt/trn_rl_repo" not in sys.path:
        sys.path.insert(0, "/opt/trn_rl_repo")
    from concourse import bass_utils

    n = node_embeddings.shape[0]
    src = np.asarray(adjacency_lists)[:, 0]
    dst = np.asarray(adjacency_lists)[:, 1]
    rt = route_edges(src, dst, n)
    rt_sig = (rt["tot"], tuple(rt["S"].tolist()))
    nc = _get_program(n, rt_sig, rt)
    in_maps = _prepare_inputs(node_embeddings, adjacency_lists,
                              W1, b1, W2, b2, rt)
    res = bass_utils.run_bass_kernel_spmd(
        nc, in_maps, core_ids=list(range(CORES)), trace=trace)
    out = np.concatenate([res.results[k]["out"] for k in range(CORES)], axis=0)
    kernel.last_result = res
    return out


# revision 17
# speedup vs baseline: 1.0017x; 1.0017x over previous
"""Trainium2 Bass kernel for a 2-layer GCN (segment-sum aggregation).

out = softmax( A @ relu(A @ h @ W1 + b1) @ W2 + b2 ),  A = adjacency (+self loops)

Strategy (8 NeuronCores, node/data parallel):
  - Nodes sharded by range: core k owns nodes [k*12500, (k+1)*12500).
  - Edges routed (on host) to the core owning their dst node, grouped by
    (128-node dst window, src%4 phase). Slot sizes are the max count over the
    8 cores rounded to 32 (SPMD: one program serves all cores); slots are
    packed contiguously per (group, phase) run, so chunks may straddle slot
    boundaries (the one-hot masks foreign positions with -1000).
  - Layer 1 on device: dma_gather of h[src] rows (256B) from an HBM table
    viewed as [N/4, 1024B] super-rows (dodges the signed-int16 index limit);
    per 128-edge chunk a one-hot [edge x node] matrix (built batched, one
    DVE tensor_tensor is_equal per window) and the TensorEngine accumulates
    aggT = sum featsT @ onehot into PSUM per window (segment-sum).
    Self loops are a dense add.  Then x1T = relu(W1.T @ aggT + b1) and
    y = x1T.T @ W2 (transform BEFORE layer-2 aggregation: 128 -> 40 dims).
  - AllGather of per-core y slices -> full y table (same super-row layout).
    b2 is folded into the local yown copy while the AllGather runs.
  - Layer 2: same gathers/one-hots vs the y table, orientation flipped to
    give node-major [128,64] windows; + self loop (incl b2); softmax on chip
    (Exp with accum_out row-sum + fast approx reciprocal).
"""

import math
import numpy as np

D = 64          # input feature dim (one gather row = 256B)
HID = 128
C = 40
CORES = 8
WIN = 128       # dst window (nodes per one-hot matmul window)
NPHASE = 4      # src mod-4 phases (int16 gather index reach)
GROUP = 4       # windows per gather instruction group
SLOT_GRAN = 32  # slot size granularity (num_idxs must be %16)
MAXIDX = 1024   # max idxs per dma_gather (64 desc/engine = 1 packet)
# AllGather split points (stage-A group index after which each part fires).
# Parts must be window-aligned; the last part is fired after stage A.
CC_GROUPS = (12, 20)


def _cc_parts(nloc):
    """Per-part (row_lo, row_hi, sr, y4_row_off) for the split AllGather."""
    bounds = [0] + [(g + 1) * GROUP * WIN for g in CC_GROUPS] + [nloc]
    parts = []
    off = 0
    for lo, hi in zip(bounds[:-1], bounds[1:]):
        sr = (hi - lo) // NPHASE
        parts.append((lo, hi, sr, off))
        off += CORES * sr
    return parts


def _remap_idx(v, nloc):
    """Map global super-row v (= core*nloc/4 + r) to its row in the
    part-major y4 table produced by the split AllGather."""
    srl = nloc // NPHASE
    c = v // srl
    r = v % srl
    out = np.zeros_like(v)
    for lo, hi, sr, off in _cc_parts(nloc):
        lo4, hi4 = lo // NPHASE, hi // NPHASE
        m = (r >= lo4) & (r < hi4)
        out[m] = off + c[m] * sr + (r[m] - lo4)
    return out


# ----------------------------------------------------------------------------
# Host-side routing
# ----------------------------------------------------------------------------

def route_edges(src, dst, n_nodes, cores=CORES):
    """Group edges by (core, window, phase) into contiguous runs per
    (group, phase).  Slot sizes are shared across cores (max, %32); the
    edge->position layout is identical for all cores (SPMD program)."""
    nloc = n_nodes // cores
    nw = math.ceil(nloc / WIN)
    ngroups = math.ceil(nw / GROUP)
    src = src.astype(np.int64)
    dst = dst.astype(np.int64)
    core = dst // nloc
    dloc = dst % nloc
    w = dloc // WIN
    phi = src % NPHASE
    key = (w * NPHASE + phi).astype(np.int64)   # per-core key in [0, nw*4)

    counts = np.zeros((cores, nw * NPHASE), np.int64)
    for k in range(cores):
        counts[k] = np.bincount(key[core == k], minlength=nw * NPHASE)
    nmax = counts.max(axis=0)                   # [nw*4]
    S = ((nmax + SLOT_GRAN - 1) // SLOT_GRAN * SLOT_GRAN).astype(np.int64)
    # ensure every window has at least one slot (self-loop-only windows)
    for wi in range(nw):
        if S[wi * NPHASE:(wi + 1) * NPHASE].sum() == 0:
            S[wi * NPHASE] = SLOT_GRAN

    def group_windows(g):
        return range(g * GROUP, min((g + 1) * GROUP, nw))

    # --- run layout: runs are (g, j) with j = p>>1 (phases 2j, 2j+1 share
    # the gather in_ap column), slots contiguous within the run ---
    offs = np.zeros(nw * NPHASE, np.int64)   # slot -> global idx-stream offset
    a_rel = np.zeros(nw * NPHASE, np.int64)  # slot -> offset within its run
    slot_col0 = np.zeros(nw * NPHASE, np.int64)  # slot -> run chunk col base
    run_len = {}                             # (g, j) -> total idxs
    run_off = {}                             # (g, j) -> global idx offset
    col0 = {}                                # (g, j) -> first chunk col in gbuf
    gchunks = []                             # chunk cols per group buffer
    tot = 0
    for g in range(ngroups):
        col = 0
        for j in range(NPHASE // 2):
            run_off[(g, j)] = tot
            col0[(g, j)] = col
            acc = 0
            for p in (2 * j, 2 * j + 1):
                for wi in group_windows(g):
                    s = wi * NPHASE + p
                    offs[s] = tot + acc
                    a_rel[s] = acc
                    slot_col0[s] = col
                    acc += int(S[s])
            run_len[(g, j)] = acc
            tot += acc
            col += (acc + WIN - 1) // WIN
        gchunks.append(col)

    # --- per-window one-hot column allocation (all 4 phases contiguous) ---
    # chunks[(w)] = list of (dstcol, gbuf_chunk_col, half) in matmul order
    dcol = 0
    dcol0 = np.zeros(nw, np.int64)
    wspan = np.zeros(nw, np.int64)
    chunks = {w: [] for w in range(nw)}
    # map: for each dst column, (slot, c_global) to fill host-side values
    dstcol_fill = []   # list of (slot, run-relative chunk index c)
    for g in range(ngroups):
        for wi in group_windows(g):
            dcol0[wi] = dcol
            for p in range(NPHASE):
                s = wi * NPHASE + p
                if S[s] == 0:
                    continue
                a = int(a_rel[s])
                c_lo = a // WIN
                c_hi = (a + int(S[s]) + WIN - 1) // WIN
                for c in range(c_lo, c_hi):
                    chunks[wi].append((dcol, int(slot_col0[s]) + c, p & 1))
                    dstcol_fill.append((s, c))
                    dcol += 1
            wspan[wi] = dcol - dcol0[wi]
    ndcols = dcol

    # --- per-core streams ---
    idx_streams, dst_streams = [], []
    for k in range(cores):
        sel = core == k
        kk = key[sel]
        sidx = np.argsort(kk, kind="stable")
        kk_s = kk[sidx]
        occ = np.arange(len(kk_s)) - np.repeat(
            np.r_[0, np.cumsum(np.bincount(kk_s, minlength=nw * NPHASE))[:-1]][kk_s], 1)
        pos = offs[kk_s] + occ
        idx = np.zeros(tot, np.int16)           # pad: super-row 0 (valid)
        idx[pos] = (src[sel][sidx] >> 2).astype(np.int16)
        # dst one-hot values, laid out per allocated dst column
        dsl_slot = np.full(tot, -1000.0, np.float32)  # per idx position
        dsl_slot[pos] = (dloc[sel][sidx] % WIN).astype(np.float32)
        dstv = np.full((ndcols, WIN), -1000.0, np.float32)
        for dc, (s, c) in enumerate(dstcol_fill):
            a = int(a_rel[s])
            g = -1  # recover run offset
            # run-relative position range of this chunk: [c*WIN, (c+1)*WIN)
            # slot covers run-relative [a, a+S[s])
            lo = max(c * WIN, a)
            hi = min((c + 1) * WIN, a + int(S[s]))
            # global idx positions: offs[s] + (i - a) for i in [lo, hi)
            qs = np.arange(lo, hi)
            dstv[dc, qs % WIN] = dsl_slot[offs[s] + (qs - a)]
        idx_streams.append(idx)
        dst_streams.append(dstv.T.copy())       # [WIN, ndcols]
    return dict(S=S, tot=tot, nw=nw, nloc=nloc, ngroups=ngroups,
                run_len=run_len, run_off=run_off, col0=col0, gchunks=gchunks,
                dcol0=dcol0, wspan=wspan, chunks=chunks, ndcols=ndcols,
                idx=idx_streams, dst=dst_streams)


# ----------------------------------------------------------------------------
# Bass program
# ----------------------------------------------------------------------------

def build_program(n_nodes, rt, do_cc=True, split_cc=True):
    import concourse.bass as bass
    import concourse.mybir as mybir
    import concourse.bacc as bacc
    from concourse import tile

    f32 = mybir.dt.float32
    bf16 = mybir.dt.float16
    i16 = mybir.dt.int16
    tot, nw, nloc, ngroups = rt["tot"], rt["nw"], rt["nloc"], rt["ngroups"]
    run_len, run_off, col0 = rt["run_len"], rt["run_off"], rt["col0"]
    gchunks, dcol0, wspan, chunks = (rt["gchunks"], rt["dcol0"], rt["wspan"],
                                     rt["chunks"])
    ndcols = rt["ndcols"]
    nsup = n_nodes // NPHASE               # super-rows in gather tables
    nlocp = nw * WIN                       # padded local node count
    last_rows = nloc - (nw - 1) * WIN      # rows in the last window
    gcmax = max(gchunks)                   # gather buffer chunk cols
    spanmax = int(max(wspan))              # widest per-window one-hot

    nc = bacc.Bacc(None, target_bir_lowering=False, debug=False,
                   num_swdge_queues=4)

    h4 = nc.declare_dram_parameter("h4", [nsup, NPHASE * D], bf16, False)
    hTo = nc.declare_dram_parameter("hTo", [D, nlocp], f32, False)
    W1d = nc.declare_dram_parameter("W1", [D, HID], f32, False)
    b1d = nc.declare_dram_parameter("b1", [HID, 1], f32, False)
    W2d = nc.declare_dram_parameter("W2p", [HID, D], f32, False)
    b2d = nc.declare_dram_parameter("b2b", [WIN, D], f32, False)
    idxd = nc.declare_dram_parameter("idx", [128, tot // 16], i16, False)
    idxd2 = nc.declare_dram_parameter("idx2", [128, tot // 16], i16, False)
    dstd = nc.declare_dram_parameter("dstf", [WIN, ndcols], bf16, False)
    iotad = nc.declare_dram_parameter("iota", [WIN, spanmax * WIN], bf16, False)
    outd = nc.declare_dram_parameter("out", [nloc, C], f32, True)

    cc_in = nc.dram_tensor("cc_in", [nloc, D], bf16)
    y4 = nc.dram_tensor("y4", [CORES * nloc // NPHASE, NPHASE * D], bf16,
                        addr_space="Shared")

    def group_windows(g):
        return range(g * GROUP, min((g + 1) * GROUP, nw))

    Relu = mybir.ActivationFunctionType.Relu
    Exp = mybir.ActivationFunctionType.Exp
    add_op = mybir.AluOpType.add
    eq_op = mybir.AluOpType.is_equal

    with tile.TileContext(nc) as tc:
        import contextlib
        with contextlib.ExitStack() as ctx:
            cpool = ctx.enter_context(tc.tile_pool(name="const", bufs=1))
            ypool = ctx.enter_context(tc.tile_pool(name="yown", bufs=1))

            idx_sb = cpool.tile([128, tot // 16], i16)
            dst_sb = cpool.tile([WIN, ndcols], bf16)
            iota_sb = cpool.tile([WIN, spanmax * WIN], bf16)
            hTo_sb = cpool.tile([D, nlocp], f32)
            W1_sb = cpool.tile([D, HID], f32)
            b1_sb = cpool.tile([HID, 1], f32)
            W2_sb = cpool.tile([HID, D], f32)
            b2_sb = cpool.tile([WIN, D], f32)
            yown = ypool.tile([WIN, nw * D], f32)

            nc.sync.dma_start(idx_sb[:], idxd[:])
            nc.sync.dma_start(dst_sb[:], dstd[:])
            nc.sync.dma_start(iota_sb[:], iotad[:])
            nc.sync.dma_start(hTo_sb[:], hTo[:])
            nc.sync.dma_start(W1_sb[:], W1d[:])
            nc.sync.dma_start(b1_sb[:], b1d[:])
            nc.sync.dma_start(W2_sb[:], W2d[:])
            nc.sync.dma_start(b2_sb[:], b2d[:])

            qctr = [0]      # round-robin SWDGE queue (4 Q7 core pairs)

            def issue_gathers(g, gt, table):
                for j in range(NPHASE // 2):
                    n = run_len[(g, j)]
                    if n == 0:
                        continue
                    o = run_off[(g, j)]
                    c0 = col0[(g, j)]
                    for s0 in range(0, n, MAXIDX):
                        ni = min(MAXIDX, n - s0)
                        cc0 = c0 + s0 // WIN
                        nch_i = (ni + WIN - 1) // WIN
                        oo = o + s0
                        nc.gpsimd.dma_gather(
                            out_ap=gt[:, cc0 * 2 * D:(cc0 + nch_i) * 2 * D]
                                .rearrange("p (c f) -> p c f", f=2 * D),
                            in_ap=table[:, j * 2 * D:(j + 1) * 2 * D],
                            idxs_ap=idx_sb[:, oo // 16: (oo + ni) // 16],
                            num_idxs=ni,
                            num_idxs_reg=ni,
                            elem_size=2 * D,
                            elem_step=NPHASE * D,
                            queue_num=qctr[0] % 4,
                        )
                        qctr[0] += 1

            def build_onehot(ohpool, wi):
                span = int(wspan[wi])
                d0 = int(dcol0[wi])
                oh = ohpool.tile([WIN, spanmax * WIN], bf16)
                nc.vector.tensor_tensor(
                    oh[:, :span * WIN].rearrange("p (c f) -> p c f", f=WIN),
                    iota_sb[:, :span * WIN].rearrange("p (c f) -> p c f", f=WIN),
                    dst_sb[:, d0:d0 + span].unsqueeze(2)
                        .broadcast_to((WIN, span, WIN)),
                    eq_op)
                return oh

            # split AllGather: early parts fire mid stage-A so their wire
            # time hides under layer-1 compute.  The y4 table is laid out
            # part-major (each part's AllGather output is contiguous); the
            # layer-2 idx stream (idx2) is remapped host-side to match.
            cc_parts = _cc_parts(nloc)
            cc_after = {CC_GROUPS[i]: cc_parts[i] for i in range(len(CC_GROUPS))}

            def fire_cc(part):
                lo, hi, sr, off = part
                nc.gpsimd.collective_compute(
                    "AllGather", mybir.AluOpType.bypass,
                    replica_groups=[list(range(CORES))],
                    ins=[cc_in.ap()[lo:hi, :]],
                    outs=[y4.ap()[off:off + CORES * sr, :]])

            # ---------------- stage A: layer 1 ----------------
            with contextlib.ExitStack() as sa:
                gpool = sa.enter_context(tc.tile_pool(name="gatherA", bufs=4))
                ohpool = sa.enter_context(tc.tile_pool(name="ohA", bufs=8))
                aggpool = sa.enter_context(tc.tile_pool(name="aggT", bufs=4))
                xpool = sa.enter_context(tc.tile_pool(name="x1", bufs=4))
                psA = sa.enter_context(
                    tc.tile_pool(name="psA", bufs=3, space="PSUM"))
                psB = sa.enter_context(
                    tc.tile_pool(name="psB", bufs=2, space="PSUM"))
                psC = sa.enter_context(
                    tc.tile_pool(name="psC", bufs=2, space="PSUM"))

                for g in range(ngroups):
                    gt = gpool.tile([WIN, gcmax * 2 * D], bf16, tag="gbuf")
                    if g < 4:
                        nc.vector.memset(gt[:], 0.0)
                    issue_gathers(g, gt, h4)
                    for wi in group_windows(g):
                        ch = chunks[wi]
                        oh = build_onehot(ohpool, wi)
                        d0 = int(dcol0[wi])
                        ps = psA.tile([D, WIN], f32)
                        for i, (dc, gcol, half) in enumerate(ch):
                            c0 = gcol * 2 * D + half * D
                            ci = dc - d0
                            nc.tensor.matmul(
                                ps[:], gt[:, c0:c0 + D],
                                oh[:, ci * WIN:(ci + 1) * WIN],
                                start=(i == 0), stop=(i == len(ch) - 1))
                        aggT = aggpool.tile([D, WIN], f32)
                        nc.vector.tensor_tensor(
                            aggT[:], ps[:], hTo_sb[:, wi * WIN:(wi + 1) * WIN],
                            add_op)
                        ps2 = psB.tile([HID, WIN], f32)
                        nc.tensor.matmul(ps2[:], W1_sb[:], aggT[:])
                        x1 = xpool.tile([HID, WIN], f32)
                        nc.scalar.activation(x1[:], ps2[:], Relu,
                                             bias=b1_sb[:, 0:1])
                        ps3 = psC.tile([WIN, D], f32)
                        nc.tensor.matmul(ps3[:], x1[:], W2_sb[:])
                        nc.scalar.copy(yown[:, wi * D:(wi + 1) * D], ps3[:])
                        ybf = xpool.tile([WIN, D], bf16, tag="ybf")
                        nc.scalar.copy(ybf[:], ps3[:])
                        rows = last_rows if wi == nw - 1 else WIN
                        nc.sync.dma_start(
                            cc_in[wi * WIN: wi * WIN + rows, :],
                            ybf[:rows, :])
                    if do_cc and split_cc and g in cc_after:
                        fire_cc(cc_after[g])

            # ---------------- all-gather of y ----------------
            if do_cc:
                if split_cc:
                    fire_cc(cc_parts[-1])
                else:
                    nc.gpsimd.collective_compute(
                        "AllGather", mybir.AluOpType.bypass,
                        replica_groups=[list(range(CORES))],
                        ins=[cc_in.ap().opt()], outs=[y4.ap().opt()])

            # swap in the layer-2 index stream (remapped for the part-major
            # y4 layout); runs in the AllGather bubble
            nc.sync.dma_start(idx_sb[:], idxd2[:])

            # fold b2 into the self-loop copy while the AllGather runs
            nc.vector.tensor_tensor(
                yown[:].rearrange("p (w d) -> p w d", d=D),
                yown[:].rearrange("p (w d) -> p w d", d=D),
                b2_sb[:].unsqueeze(1).broadcast_to((WIN, nw, D)),
                add_op)

            # ---------------- stage C: layer 2 ----------------
            with contextlib.ExitStack() as sc:
                gpool = sc.enter_context(tc.tile_pool(name="gatherC", bufs=4))
                ohpool = sc.enter_context(tc.tile_pool(name="ohC", bufs=8))
                spool = sc.enter_context(tc.tile_pool(name="smax", bufs=4))
                opool = sc.enter_context(tc.tile_pool(name="outp", bufs=3))
                psD = sc.enter_context(
                    tc.tile_pool(name="psD", bufs=4, space="PSUM"))

                for g in range(ngroups):
                    gt = gpool.tile([WIN, gcmax * 2 * D], bf16, tag="gbufC")
                    if g < 4:
                        nc.vector.memset(gt[:], 0.0)
                    issue_gathers(g, gt, y4)
                    for wi in group_windows(g):
                        ch = chunks[wi]
                        oh = build_onehot(ohpool, wi)
                        d0 = int(dcol0[wi])
                        ps = psD.tile([WIN, D], f32)
                        for i, (dc, gcol, half) in enumerate(ch):
                            c0 = gcol * 2 * D + half * D
                            ci = dc - d0
                            nc.tensor.matmul(
                                ps[:], oh[:, ci * WIN:(ci + 1) * WIN],
                                gt[:, c0:c0 + D],
                                start=(i == 0), stop=(i == len(ch) - 1))
                        t1 = spool.tile([WIN, D], f32, tag="t1")
                        nc.vector.tensor_tensor(
                            t1[:], ps[:], yown[:, wi * D:(wi + 1) * D], add_op)
                        mx = spool.tile([WIN, 1], f32, tag="mx")
                        nc.vector.tensor_reduce(
                            mx[:], t1[:, :C], mybir.AxisListType.X,
                            mybir.AluOpType.max, negate=True)
                        e = spool.tile([WIN, C], f32, tag="e")
                        sm = spool.tile([WIN, 1], f32, tag="sm")
                        nc.scalar.activation(e[:], t1[:, :C], Exp,
                                             bias=mx[:, 0:1],
                                             accum_out=sm[:, 0:1])
                        ri = spool.tile([WIN, 1], f32, tag="ri")
                        nc.vector.reciprocal_approx_fast(ri[:], sm[:])
                        o = opool.tile([WIN, C], f32)
                        nc.scalar.activation(
                            o[:], e[:], mybir.ActivationFunctionType.Identity,
                            scale=ri[:, 0:1])
                        rows = last_rows if wi == nw - 1 else WIN
                        nc.sync.dma_start(
                            outd[wi * WIN: wi * WIN + rows, :], o[:rows, :])

    nc.finalize()
    return nc


# ----------------------------------------------------------------------------
# Entry point
# ----------------------------------------------------------------------------

def _prepare_inputs(node_embeddings, adjacency_lists, W1, b1, W2, b2, rt):
    n, d = node_embeddings.shape
    nloc, nw = rt["nloc"], rt["nw"]
    nlocp = nw * WIN
    spanmax = int(max(rt["wspan"]))
    bf = np.float16
    h = np.ascontiguousarray(node_embeddings, np.float32)
    h4 = h.astype(bf).reshape(n // NPHASE, NPHASE * d)
    W2p = np.zeros((HID, D), np.float32)
    W2p[:, :C] = W2
    b2b = np.tile(np.pad(b2.astype(np.float32), (0, D - C)), (WIN, 1))
    iota = np.tile(np.arange(WIN, dtype=np.float32), (WIN, spanmax))
    in_maps = []
    for k in range(CORES):
        hTo = np.zeros((d, nlocp), np.float32)
        hTo[:, :nloc] = h[k * nloc:(k + 1) * nloc].T
        in_maps.append({
            "h4": h4,
            "hTo": hTo,
            "W1": np.ascontiguousarray(W1, np.float32),
            "b1": np.ascontiguousarray(b1, np.float32).reshape(HID, 1),
            "W2p": W2p,
            "b2b": b2b,
            "idx": np.tile(rt["idx"][k].reshape(-1, 16).T, (8, 1)).copy(),
            "idx2": np.tile(
                _remap_idx(rt["idx"][k].astype(np.int64), nloc)
                .astype(np.int16).reshape(-1, 16).T, (8, 1)).copy(),
            "dstf": np.ascontiguousarray(rt["dst"][k]).astype(bf),
            "iota": iota.astype(bf),
            "out": np.zeros((nloc, C), np.float32),
        })
    return in_maps


_CACHE = {}


def _get_program(n_nodes, rt_sig, rt):
    key = (n_nodes, rt_sig)
    if key not in _CACHE:
        _CACHE[key] = build_program(n_nodes, rt)
    return _CACHE[key]


def kernel(node_embeddings, adjacency_lists, W1, b1, W2, b2, trace=False):
    import sys
    if "/opt/trn_rl_repo" not in sys.path:
        sys.path.insert(0, "/opt/trn_rl_repo")
    from concourse import bass_utils

    n = node_embeddings.shape[0]
    src = np.asarray(adjacency_lists)[:, 0]
    dst = np.asarray(adjacency_lists)[:, 1]
    rt = route_edges(src, dst, n)
    rt_sig = (rt["tot"], tuple(rt["S"].tolist()))
    nc = _get_program(n, rt_sig, rt)
    in_maps = _prepare_inputs(node_embeddings, adjacency_lists,
                              W1, b1, W2, b2, rt)
    res = bass_utils.run_bass_kernel_spmd(
        nc, in_maps, core_ids=list(range(CORES)), trace=trace)
    out = np.concatenate([res.results[k]["out"] for k in range(CORES)], axis=0)
    kernel.last_result = res
    return out


# revision 18
# speedup vs baseline: 1.2590x; 1.2569x over previous
"""Trainium2 Bass kernel for a 2-layer GCN (segment-sum aggregation).

out = softmax( A @ relu(A @ h @ W1 + b1) @ W2 + b2 ),  A = adjacency (+self loops)

Strategy (8 NeuronCores, node/data parallel):
  - Nodes sharded by range: core k owns nodes [k*12500, (k+1)*12500).
  - Edges routed (on host) to the core owning their dst node, grouped by
    (128-node dst window, src%4 phase). Slot sizes are the max count over the
    8 cores rounded to 32 (SPMD: one program serves all cores); slots are
    packed contiguously per (group, phase) run, so chunks may straddle slot
    boundaries (the one-hot masks foreign positions with -1000).
  - Layer 1 on device: dma_gather of h[src] rows (256B) from an HBM table
    viewed as [N/4, 1024B] super-rows (dodges the signed-int16 index limit);
    per 128-edge chunk a one-hot [edge x node] matrix (built batched, one
    DVE tensor_tensor is_equal per window) and the TensorEngine accumulates
    aggT = sum featsT @ onehot into PSUM per window (segment-sum).
    Self loops are a dense add.  Then x1T = relu(W1.T @ aggT + b1) and
    y = x1T.T @ W2 (transform BEFORE layer-2 aggregation: 128 -> 40 dims).
  - AllGather of per-core y slices -> full y table (same super-row layout).
    b2 is folded into the local yown copy while the AllGather runs.
  - Layer 2: same gathers/one-hots vs the y table, orientation flipped to
    give node-major [128,64] windows; + self loop (incl b2); softmax on chip
    (Exp with accum_out row-sum + fast approx reciprocal).
"""

import math
import numpy as np

D = 64          # input feature dim (one gather row = 256B)
HID = 128
C = 40
CORES = 8
WIN = 128       # dst window (nodes per one-hot matmul window)
NPHASE = 4      # src mod-4 phases (int16 gather index reach)
GROUP = 4       # windows per gather instruction group
SLOT_GRAN = 32  # slot size granularity (num_idxs must be %16)
MAXIDX = 1024   # max idxs per dma_gather (64 desc/engine = 1 packet)
# AllGather split points (stage-A group index after which each part fires).
# Parts must be window-aligned; the last part is fired after stage A.
CC_GROUPS = (12, 20)


def _cc_parts(nloc):
    """Per-part (row_lo, row_hi, sr, y4_row_off) for the split AllGather."""
    bounds = [0] + [(g + 1) * GROUP * WIN for g in CC_GROUPS] + [nloc]
    parts = []
    off = 0
    for lo, hi in zip(bounds[:-1], bounds[1:]):
        sr = (hi - lo) // NPHASE
        parts.append((lo, hi, sr, off))
        off += CORES * sr
    return parts


def _remap_idx(v, nloc):
    """Map global super-row v (= core*nloc/4 + r) to its row in the
    part-major y4 table produced by the split AllGather."""
    srl = nloc // NPHASE
    c = v // srl
    r = v % srl
    out = np.zeros_like(v)
    for lo, hi, sr, off in _cc_parts(nloc):
        lo4, hi4 = lo // NPHASE, hi // NPHASE
        m = (r >= lo4) & (r < hi4)
        out[m] = off + c[m] * sr + (r[m] - lo4)
    return out


# ----------------------------------------------------------------------------
# Host-side routing
# ----------------------------------------------------------------------------

def route_edges(src, dst, n_nodes, cores=CORES):
    """Group edges by (core, window, phase) into contiguous runs per
    (group, phase).  Slot sizes are shared across cores (max, %32); the
    edge->position layout is identical for all cores (SPMD program)."""
    nloc = n_nodes // cores
    nw = math.ceil(nloc / WIN)
    ngroups = math.ceil(nw / GROUP)
    src = src.astype(np.int64)
    dst = dst.astype(np.int64)
    core = dst // nloc
    dloc = dst % nloc
    w = dloc // WIN
    phi = src % NPHASE
    key = (w * NPHASE + phi).astype(np.int64)   # per-core key in [0, nw*4)

    counts = np.zeros((cores, nw * NPHASE), np.int64)
    for k in range(cores):
        counts[k] = np.bincount(key[core == k], minlength=nw * NPHASE)
    nmax = counts.max(axis=0)                   # [nw*4]
    S = ((nmax + SLOT_GRAN - 1) // SLOT_GRAN * SLOT_GRAN).astype(np.int64)
    # ensure every window has at least one slot (self-loop-only windows)
    for wi in range(nw):
        if S[wi * NPHASE:(wi + 1) * NPHASE].sum() == 0:
            S[wi * NPHASE] = SLOT_GRAN

    def group_windows(g):
        return range(g * GROUP, min((g + 1) * GROUP, nw))

    # --- run layout: runs are (g, j) with j = p>>1 (phases 2j, 2j+1 share
    # the gather in_ap column), slots contiguous within the run ---
    offs = np.zeros(nw * NPHASE, np.int64)   # slot -> global idx-stream offset
    a_rel = np.zeros(nw * NPHASE, np.int64)  # slot -> offset within its run
    slot_col0 = np.zeros(nw * NPHASE, np.int64)  # slot -> run chunk col base
    run_len = {}                             # (g, j) -> total idxs
    run_off = {}                             # (g, j) -> global idx offset
    col0 = {}                                # (g, j) -> first chunk col in gbuf
    gchunks = []                             # chunk cols per group buffer
    tot = 0
    for g in range(ngroups):
        col = 0
        for j in range(NPHASE // 2):
            run_off[(g, j)] = tot
            col0[(g, j)] = col
            acc = 0
            for p in (2 * j, 2 * j + 1):
                for wi in group_windows(g):
                    s = wi * NPHASE + p
                    offs[s] = tot + acc
                    a_rel[s] = acc
                    slot_col0[s] = col
                    acc += int(S[s])
            run_len[(g, j)] = acc
            tot += acc
            col += (acc + WIN - 1) // WIN
        gchunks.append(col)

    # --- per-window one-hot column allocation (all 4 phases contiguous) ---
    # chunks[(w)] = list of (dstcol, gbuf_chunk_col, half) in matmul order
    dcol = 0
    dcol0 = np.zeros(nw, np.int64)
    wspan = np.zeros(nw, np.int64)
    chunks = {w: [] for w in range(nw)}
    # map: for each dst column, (slot, c_global) to fill host-side values
    dstcol_fill = []   # list of (slot, run-relative chunk index c)
    for g in range(ngroups):
        for wi in group_windows(g):
            dcol0[wi] = dcol
            for p in range(NPHASE):
                s = wi * NPHASE + p
                if S[s] == 0:
                    continue
                a = int(a_rel[s])
                c_lo = a // WIN
                c_hi = (a + int(S[s]) + WIN - 1) // WIN
                for c in range(c_lo, c_hi):
                    chunks[wi].append((dcol, int(slot_col0[s]) + c, p & 1))
                    dstcol_fill.append((s, c))
                    dcol += 1
            wspan[wi] = dcol - dcol0[wi]
    ndcols = dcol

    # --- per-core streams ---
    idx_streams, dst_streams = [], []
    for k in range(cores):
        sel = core == k
        kk = key[sel]
        sidx = np.argsort(kk, kind="stable")
        kk_s = kk[sidx]
        occ = np.arange(len(kk_s)) - np.repeat(
            np.r_[0, np.cumsum(np.bincount(kk_s, minlength=nw * NPHASE))[:-1]][kk_s], 1)
        pos = offs[kk_s] + occ
        idx = np.zeros(tot, np.int16)           # pad: super-row 0 (valid)
        idx[pos] = (src[sel][sidx] >> 2).astype(np.int16)
        # dst one-hot values, laid out per allocated dst column
        dsl_slot = np.full(tot, -1000.0, np.float32)  # per idx position
        dsl_slot[pos] = (dloc[sel][sidx] % WIN).astype(np.float32)
        dstv = np.full((ndcols, WIN), -1000.0, np.float32)
        for dc, (s, c) in enumerate(dstcol_fill):
            a = int(a_rel[s])
            g = -1  # recover run offset
            # run-relative position range of this chunk: [c*WIN, (c+1)*WIN)
            # slot covers run-relative [a, a+S[s])
            lo = max(c * WIN, a)
            hi = min((c + 1) * WIN, a + int(S[s]))
            # global idx positions: offs[s] + (i - a) for i in [lo, hi)
            qs = np.arange(lo, hi)
            dstv[dc, qs % WIN] = dsl_slot[offs[s] + (qs - a)]
        idx_streams.append(idx)
        dst_streams.append(dstv.T.copy())       # [WIN, ndcols]
    return dict(S=S, tot=tot, nw=nw, nloc=nloc, ngroups=ngroups,
                run_len=run_len, run_off=run_off, col0=col0, gchunks=gchunks,
                dcol0=dcol0, wspan=wspan, chunks=chunks, ndcols=ndcols,
                idx=idx_streams, dst=dst_streams)


# ----------------------------------------------------------------------------
# Bass program
# ----------------------------------------------------------------------------

def build_program(n_nodes, rt, do_cc=True, split_cc=True):
    import concourse.bass as bass
    import concourse.mybir as mybir
    import concourse.bacc as bacc
    from concourse import tile

    f32 = mybir.dt.float32
    bf16 = mybir.dt.float16
    i16 = mybir.dt.int16
    tot, nw, nloc, ngroups = rt["tot"], rt["nw"], rt["nloc"], rt["ngroups"]
    run_len, run_off, col0 = rt["run_len"], rt["run_off"], rt["col0"]
    gchunks, dcol0, wspan, chunks = (rt["gchunks"], rt["dcol0"], rt["wspan"],
                                     rt["chunks"])
    ndcols = rt["ndcols"]
    nsup = n_nodes // NPHASE               # super-rows in gather tables
    nlocp = nw * WIN                       # padded local node count
    last_rows = nloc - (nw - 1) * WIN      # rows in the last window
    gcmax = max(gchunks)                   # gather buffer chunk cols
    spanmax = int(max(wspan))              # widest per-window one-hot

    nc = bacc.Bacc(None, target_bir_lowering=False, debug=False,
                   num_swdge_queues=4)

    h4 = nc.declare_dram_parameter("h4", [nsup, NPHASE * D], bf16, False)
    hTo = nc.declare_dram_parameter("hTo", [D, nlocp], f32, False)
    W1d = nc.declare_dram_parameter("W1", [D, HID], f32, False)
    b1d = nc.declare_dram_parameter("b1", [HID, 1], f32, False)
    W2d = nc.declare_dram_parameter("W2p", [HID, D], f32, False)
    b2d = nc.declare_dram_parameter("b2b", [WIN, D], f32, False)
    idxd = nc.declare_dram_parameter("idx", [128, tot // 16], i16, False)
    idxd2 = nc.declare_dram_parameter("idx2", [128, tot // 16], i16, False)
    dstd = nc.declare_dram_parameter("dstf", [WIN, ndcols], bf16, False)
    iotad = nc.declare_dram_parameter("iota", [WIN, spanmax * WIN], bf16, False)
    outd = nc.declare_dram_parameter("out", [nloc, C], f32, True)

    cc_in = nc.dram_tensor("cc_in", [nloc, D], bf16)
    y4 = nc.dram_tensor("y4", [CORES * nloc // NPHASE, NPHASE * D], bf16,
                        addr_space="Shared")

    def group_windows(g):
        return range(g * GROUP, min((g + 1) * GROUP, nw))

    Relu = mybir.ActivationFunctionType.Relu
    Exp = mybir.ActivationFunctionType.Exp
    add_op = mybir.AluOpType.add
    eq_op = mybir.AluOpType.is_equal

    with tile.TileContext(nc) as tc:
        import contextlib
        with contextlib.ExitStack() as ctx:
            cpool = ctx.enter_context(tc.tile_pool(name="const", bufs=1))
            ypool = ctx.enter_context(tc.tile_pool(name="yown", bufs=1))

            idx_sb = cpool.tile([128, tot // 16], i16)
            dst_sb = cpool.tile([WIN, ndcols], bf16)
            iota_sb = cpool.tile([WIN, spanmax * WIN], bf16)
            hTo_sb = cpool.tile([D, nlocp], f32)
            W1_sb = cpool.tile([D, HID], f32)
            b1_sb = cpool.tile([HID, 1], f32)
            W2_sb = cpool.tile([HID, D], f32)
            b2_sb = cpool.tile([WIN, D], f32)
            yown = ypool.tile([WIN, nw * D], f32)

            nc.sync.dma_start(idx_sb[:], idxd[:])
            nc.sync.dma_start(dst_sb[:], dstd[:])
            nc.sync.dma_start(iota_sb[:], iotad[:])
            nc.sync.dma_start(hTo_sb[:], hTo[:])
            nc.sync.dma_start(W1_sb[:], W1d[:])
            nc.sync.dma_start(b1_sb[:], b1d[:])
            nc.sync.dma_start(W2_sb[:], W2d[:])
            nc.sync.dma_start(b2_sb[:], b2d[:])

            qctr = [0]      # round-robin SWDGE queue (4 Q7 core pairs)

            def issue_gathers(g, gt, table):
                for j in range(NPHASE // 2):
                    n = run_len[(g, j)]
                    if n == 0:
                        continue
                    o = run_off[(g, j)]
                    c0 = col0[(g, j)]
                    for s0 in range(0, n, MAXIDX):
                        ni = min(MAXIDX, n - s0)
                        cc0 = c0 + s0 // WIN
                        nch_i = (ni + WIN - 1) // WIN
                        oo = o + s0
                        nc.gpsimd.dma_gather(
                            out_ap=gt[:, cc0 * 2 * D:(cc0 + nch_i) * 2 * D]
                                .rearrange("p (c f) -> p c f", f=2 * D),
                            in_ap=table[:, j * 2 * D:(j + 1) * 2 * D],
                            idxs_ap=idx_sb[:, oo // 16: (oo + ni) // 16],
                            num_idxs=ni,
                            num_idxs_reg=ni,
                            elem_size=2 * D,
                            elem_step=NPHASE * D,
                            queue_num=qctr[0] % 4,
                        )
                        qctr[0] += 1

            def build_onehot(ohpool, wi):
                span = int(wspan[wi])
                d0 = int(dcol0[wi])
                oh = ohpool.tile([WIN, spanmax * WIN], bf16)
                nc.vector.tensor_tensor(
                    oh[:, :span * WIN].rearrange("p (c f) -> p c f", f=WIN),
                    iota_sb[:, :span * WIN].rearrange("p (c f) -> p c f", f=WIN),
                    dst_sb[:, d0:d0 + span].unsqueeze(2)
                        .broadcast_to((WIN, span, WIN)),
                    eq_op)
                return oh

            # split AllGather: early parts fire mid stage-A so their wire
            # time hides under layer-1 compute.  The y4 table is laid out
            # part-major (each part's AllGather output is contiguous); the
            # layer-2 idx stream (idx2) is remapped host-side to match.
            cc_parts = _cc_parts(nloc)
            cc_after = {CC_GROUPS[i]: cc_parts[i] for i in range(len(CC_GROUPS))}

            def fire_cc(part):
                lo, hi, sr, off = part
                nc.gpsimd.collective_compute(
                    "AllGather", mybir.AluOpType.bypass,
                    replica_groups=[list(range(CORES))],
                    ins=[cc_in.ap()[lo:hi, :]],
                    outs=[y4.ap()[off:off + CORES * sr, :]])

            # ---------------- stage A: layer 1 ----------------
            with contextlib.ExitStack() as sa:
                gpool = sa.enter_context(tc.tile_pool(name="gatherA", bufs=3))
                ohpool = sa.enter_context(tc.tile_pool(name="ohA", bufs=8))
                aggpool = sa.enter_context(tc.tile_pool(name="aggT", bufs=4))
                xpool = sa.enter_context(tc.tile_pool(name="x1", bufs=4))
                psA = sa.enter_context(
                    tc.tile_pool(name="psA", bufs=3, space="PSUM"))
                psB = sa.enter_context(
                    tc.tile_pool(name="psB", bufs=2, space="PSUM"))
                psC = sa.enter_context(
                    tc.tile_pool(name="psC", bufs=2, space="PSUM"))

                for g in range(ngroups):
                    gt = gpool.tile([WIN, gcmax * 2 * D], bf16, tag="gbuf")
                    if g < 3:
                        nc.vector.memset(gt[:], 0.0)
                    issue_gathers(g, gt, h4)
                    for wi in group_windows(g):
                        ch = chunks[wi]
                        oh = build_onehot(ohpool, wi)
                        d0 = int(dcol0[wi])
                        ps = psA.tile([D, WIN], f32)
                        for i, (dc, gcol, half) in enumerate(ch):
                            c0 = gcol * 2 * D + half * D
                            ci = dc - d0
                            nc.tensor.matmul(
                                ps[:], gt[:, c0:c0 + D],
                                oh[:, ci * WIN:(ci + 1) * WIN],
                                start=(i == 0), stop=(i == len(ch) - 1))
                        aggT = aggpool.tile([D, WIN], f32)
                        nc.vector.tensor_tensor(
                            aggT[:], ps[:], hTo_sb[:, wi * WIN:(wi + 1) * WIN],
                            add_op)
                        ps2 = psB.tile([HID, WIN], f32)
                        nc.tensor.matmul(ps2[:], W1_sb[:], aggT[:])
                        x1 = xpool.tile([HID, WIN], f32)
                        nc.scalar.activation(x1[:], ps2[:], Relu,
                                             bias=b1_sb[:, 0:1])
                        ps3 = psC.tile([WIN, D], f32)
                        nc.tensor.matmul(ps3[:], x1[:], W2_sb[:])
                        nc.scalar.copy(yown[:, wi * D:(wi + 1) * D], ps3[:])
                        ybf = xpool.tile([WIN, D], bf16, tag="ybf")
                        nc.scalar.copy(ybf[:], ps3[:])
                        rows = last_rows if wi == nw - 1 else WIN
                        nc.sync.dma_start(
                            cc_in[wi * WIN: wi * WIN + rows, :],
                            ybf[:rows, :])
                    if do_cc and split_cc and g in cc_after:
                        fire_cc(cc_after[g])

            # ---------------- all-gather of y ----------------
            if do_cc:
                if split_cc:
                    fire_cc(cc_parts[-1])
                else:
                    nc.gpsimd.collective_compute(
                        "AllGather", mybir.AluOpType.bypass,
                        replica_groups=[list(range(CORES))],
                        ins=[cc_in.ap().opt()], outs=[y4.ap().opt()])

            # swap in the layer-2 index stream (remapped for the part-major
            # y4 layout); runs in the AllGather bubble
            nc.sync.dma_start(idx_sb[:], idxd2[:])

            # fold b2 into the self-loop copy while the AllGather runs
            nc.vector.tensor_tensor(
                yown[:].rearrange("p (w d) -> p w d", d=D),
                yown[:].rearrange("p (w d) -> p w d", d=D),
                b2_sb[:].unsqueeze(1).broadcast_to((WIN, nw, D)),
                add_op)

            # ---------------- stage C: layer 2 ----------------
            with contextlib.ExitStack() as sc:
                gpool = sc.enter_context(tc.tile_pool(name="gatherC", bufs=3))
                ohpool = sc.enter_context(tc.tile_pool(name="ohC", bufs=8))
                spool = sc.enter_context(tc.tile_pool(name="smax", bufs=4))
                opool = sc.enter_context(tc.tile_pool(name="outp", bufs=3))
                psD = sc.enter_context(
                    tc.tile_pool(name="psD", bufs=4, space="PSUM"))

                for g in range(ngroups):
                    gt = gpool.tile([WIN, gcmax * 2 * D], bf16, tag="gbufC")
                    if g < 3:
                        nc.vector.memset(gt[:], 0.0)
                    issue_gathers(g, gt, y4)
                    for wi in group_windows(g):
                        ch = chunks[wi]
                        oh = build_onehot(ohpool, wi)
                        d0 = int(dcol0[wi])
                        ps = psD.tile([WIN, D], f32)
                        for i, (dc, gcol, half) in enumerate(ch):
                            c0 = gcol * 2 * D + half * D
                            ci = dc - d0
                            nc.tensor.matmul(
                                ps[:], oh[:, ci * WIN:(ci + 1) * WIN],
                                gt[:, c0:c0 + D],
                                start=(i == 0), stop=(i == len(ch) - 1))
                        t1 = spool.tile([WIN, D], f32, tag="t1")
                        nc.vector.tensor_tensor(
                            t1[:], ps[:], yown[:, wi * D:(wi + 1) * D], add_op)
                        mx = spool.tile([WIN, 1], f32, tag="mx")
                        nc.vector.tensor_reduce(
                            mx[:], t1[:, :C], mybir.AxisListType.X,
                            mybir.AluOpType.max, negate=True)
                        e = spool.tile([WIN, C], f32, tag="e")
                        sm = spool.tile([WIN, 1], f32, tag="sm")
                        nc.scalar.activation(e[:], t1[:, :C], Exp,
                                             bias=mx[:, 0:1],
                                             accum_out=sm[:, 0:1])
                        ri = spool.tile([WIN, 1], f32, tag="ri")
                        nc.vector.reciprocal_approx_fast(ri[:], sm[:])
                        o = opool.tile([WIN, C], f32)
                        nc.scalar.activation(
                            o[:], e[:], mybir.ActivationFunctionType.Identity,
                            scale=ri[:, 0:1])
                        rows = last_rows if wi == nw - 1 else WIN
                        nc.sync.dma_start(
                            outd[wi * WIN: wi * WIN + rows, :], o[:rows, :])

    nc.finalize()
    return nc


# ----------------------------------------------------------------------------
# Entry point
# ----------------------------------------------------------------------------

def _prepare_inputs(node_embeddings, adjacency_lists, W1, b1, W2, b2, rt):
    n, d = node_embeddings.shape
    nloc, nw = rt["nloc"], rt["nw"]
    nlocp = nw * WIN
    spanmax = int(max(rt["wspan"]))
    bf = np.float16
    h = np.ascontiguousarray(node_embeddings, np.float32)
    h4 = h.astype(bf).reshape(n // NPHASE, NPHASE * d)
    W2p = np.zeros((HID, D), np.float32)
    W2p[:, :C] = W2
    b2b = np.tile(np.pad(b2.astype(np.float32), (0, D - C)), (WIN, 1))
    iota = np.tile(np.arange(WIN, dtype=np.float32), (WIN, spanmax))
    in_maps = []
    for k in range(CORES):
        hTo = np.zeros((d, nlocp), np.float32)
        hTo[:, :nloc] = h[k * nloc:(k + 1) * nloc].T
        in_maps.append({
            "h4": h4,
            "hTo": hTo,
            "W1": np.ascontiguousarray(W1, np.float32),
            "b1": np.ascontiguousarray(b1, np.float32).reshape(HID, 1),
            "W2p": W2p,
            "b2b": b2b,
            "idx": np.tile(rt["idx"][k].reshape(-1, 16).T, (8, 1)).copy(),
            "idx2": np.tile(
                _remap_idx(rt["idx"][k].astype(np.int64), nloc)
                .astype(np.int16).reshape(-1, 16).T, (8, 1)).copy(),
            "dstf": np.ascontiguousarray(rt["dst"][k]).astype(bf),
            "iota": iota.astype(bf),
            "out": np.zeros((nloc, C), np.float32),
        })
    return in_maps


_CACHE = {}


def _get_program(n_nodes, rt_sig, rt):
    key = (n_nodes, rt_sig)
    if key not in _CACHE:
        _CACHE[key] = build_program(n_nodes, rt)
    return _CACHE[key]


def kernel(node_embeddings, adjacency_lists, W1, b1, W2, b2, trace=False):
    import sys
    if "/opt/trn_rl_repo" not in sys.path:
        sys.path.insert(0, "/opt/trn_rl_repo")
    from concourse import bass_utils

    n = node_embeddings.shape[0]
    src = np.asarray(adjacency_lists)[:, 0]
    dst = np.asarray(adjacency_lists)[:, 1]
    rt = route_edges(src, dst, n)
    rt_sig = (rt["tot"], tuple(rt["S"].tolist()))
    nc = _get_program(n, rt_sig, rt)
    in_maps = _prepare_inputs(node_embeddings, adjacency_lists,
                              W1, b1, W2, b2, rt)
    res = bass_utils.run_bass_kernel_spmd(
        nc, in_maps, core_ids=list(range(CORES)), trace=trace)
    out = np.concatenate([res.results[k]["out"] for k in range(CORES)], axis=0)
    kernel.last_result = res
    return out
